# revision 1
# baseline (speedup 1.0000x reference)
"""Trainium2 Bass kernel for nn_Attention_4063039062503.

Reference (per batch b, C=128 channels, N=4096 points):
    q = W1 @ x + b1;  k = W2 @ x + b2          # [C, N]
    s[n, m] = q[:, n] . k[:, m]                # [N, N]
    a = softmax(s, axis=m)
    out = relu(x + x @ a.T)                    # out[:, n] = x @ a[n, :]

Sharding: 8 cores, core i -> batch i//2, query half i%2 (2048 queries),
full 4096 keys local (no collectives).

Per-core plan (flash-attention style, scores never leave the chip):
  - Q/K projections + S^T score tiles [m=128, q] on TensorE in fp16
    (10-bit mantissa; simulated end-to-end error 2.7e-3 vs 2e-2 gate)
  - exp(s - 30) on ScalarE PSUM->SBUF, bf16 out (constant shift is exact
    for softmax and keeps e^s in fp32/bf16 range for scores up to ~+118)
  - O[c, q] += xT[m-tile] @ E on TensorE in bf16, fp32 PSUM accumulation
  - row-sums: DVE bf16 accumulate across m-tiles + one ones-matmul
  - normalize via reciprocal_approx_fast + fp32 broadcast-matmul, then
    residual add (fp32 x) + relu on DVE, DMA out
  - O-matmuls software-pipelined DLAG iterations behind the S/exp stream
    so their ACT-waits are pre-satisfied and the PE queue stays dense
"""
from contextlib import ExitStack

import numpy as np
import ml_dtypes

import concourse.tile as tile
from concourse import bacc, mybir
from concourse.bass_utils import run_bass_kernel_spmd

B = 4
C = 128
N = 4096            # keys per batch
NQ = 2048           # queries per core
QB = 512            # query block (PSUM bank free size)
MT = 128            # m (key) tile
N_MT = N // MT      # 32
N_QB = NQ // QB     # 4
DLAG = 3            # O-matmul lag (in m-tiles) behind the S/exp pipeline

# packed fp16 input layout: [128, XW_COLS]
XK_OFS = 0                  # x full        [128, 4096]
XQ_OFS = N                  # x query half  [128, 2048]
W1T_OFS = XQ_OFS + NQ       # W1.T          [128, 128]
W2T_OFS = W1T_OFS + C       # W2.T          [128, 128]
B1_OFS = W2T_OFS + C        # b1 column     [128, 1]
B2_OFS = B1_OFS + 1         # b2 column     [128, 1]
XW_COLS = B2_OFS + 1

F32 = mybir.dt.float32
F16 = mybir.dt.float16
BF16 = mybir.dt.bfloat16


def build_nc():
    nc = bacc.Bacc("TRN2", target_bir_lowering=False, debug=False, num_devices=8)
    xw_ext = nc.declare_dram_parameter("xw", [C, XW_COLS], F16, isOutput=False)
    xt_ext = nc.declare_dram_parameter("xt", [C, N], BF16, isOutput=False)
    xr_ext = nc.declare_dram_parameter("xr", [C, NQ + 2], F32, isOutput=False)
    out_ext = nc.declare_dram_parameter("out", [C, NQ], F32, isOutput=True)

    with ExitStack() as ctx:
        tc = ctx.enter_context(tile.TileContext(nc))
        consts = ctx.enter_context(tc.tile_pool(name="consts", bufs=1))
        sb_in = ctx.enter_context(tc.tile_pool(name="sb_in", bufs=1))
        sb_kq = ctx.enter_context(tc.tile_pool(name="sb_kq", bufs=1))
        sb_e = ctx.enter_context(tc.tile_pool(name="sb_e", bufs=1))
        sb_acc = ctx.enter_context(tc.tile_pool(name="sb_acc", bufs=2))
        sb_tail = ctx.enter_context(tc.tile_pool(name="sb_tail", bufs=2))
        ps_s = ctx.enter_context(tc.tile_pool(name="ps_s", bufs=2, space="PSUM"))
        ps_o = ctx.enter_context(tc.tile_pool(name="ps_o", bufs=2, space="PSUM"))
        ps_r = ctx.enter_context(tc.tile_pool(name="ps_r", bufs=2, space="PSUM"))


        # warm the PE's HAM clock gate (~3.4us of matmul activity) during
        # the input-DMA wait so the projection chain runs at 2.4 GHz; its
        # memset goes first so the warmup finishes before the first DMA lands
        wmm = consts.tile([C, QB], BF16, tag="wmm")
        nc.vector.memset(wmm[:], 0.0)
        for _ in range(8):
            wps = ps_s.tile([C, QB], F32, tag="s")
            nc.tensor.matmul(wps[:], wmm[:, 0:C], wmm[:], start=True, stop=True)
        ones_bf = consts.tile([C, C], BF16, tag="ones_bf")
        nc.vector.memset(ones_bf[:], 1.0)
        shift = consts.tile([C, 1], F32, tag="shift")
        nc.vector.memset(shift[:], -30.0)
        # warm the exp table early (ACT_TABLE_LOAD ~2.7us)
        warm = consts.tile([1, 16], F32, tag="warm")
        nc.vector.memset(warm[:], 0.0)
        warm_o = consts.tile([1, 16], F32, tag="warm_o")
        nc.scalar.activation(warm_o[:], warm[:], mybir.ActivationFunctionType.Exp)

        xw = sb_in.tile([C, XW_COLS], F16, tag="xw")
        xt = sb_in.tile([C, N], BF16, tag="xt")
        xr = sb_in.tile([C, NQ + 2], F32, tag="xr")
        # chunked input DMAs, ordered so the first S-matmuls start early:
        # weights, first q chunks, k chunks, rest
        nc.sync.dma_start(xw[:, W1T_OFS:XW_COLS], xw_ext[:, W1T_OFS:XW_COLS])
        nc.sync.dma_start(xr[:, NQ:NQ + 2], xr_ext[:, NQ:NQ + 2])
        # first q/k 512-col chunks land first: they gate the first exp
        nc.sync.dma_start(xw[:, XQ_OFS:XQ_OFS + QB],
                          xw_ext[:, XQ_OFS:XQ_OFS + QB])
        nc.sync.dma_start(xw[:, XK_OFS:XK_OFS + QB],
                          xw_ext[:, XK_OFS:XK_OFS + QB])
        nc.sync.dma_start(xw[:, XQ_OFS + QB:XQ_OFS + 2 * QB],
                          xw_ext[:, XQ_OFS + QB:XQ_OFS + 2 * QB])
        nc.sync.dma_start(xw[:, XK_OFS + QB:XK_OFS + 2 * QB],
                          xw_ext[:, XK_OFS + QB:XK_OFS + 2 * QB])
        nc.sync.dma_start(xw[:, XQ_OFS + 2 * QB:XQ_OFS + NQ],
                          xw_ext[:, XQ_OFS + 2 * QB:XQ_OFS + NQ])
        for j in range(1, N // (2 * QB)):
            nc.sync.dma_start(xw[:, j * 2 * QB:(j + 1) * 2 * QB],
                              xw_ext[:, j * 2 * QB:(j + 1) * 2 * QB])
        nc.sync.dma_start(xt[:], xt_ext[:])
        nc.sync.dma_start(xr[:, 0:NQ], xr_ext[:, 0:NQ])

        kt = sb_kq.tile([C, N], F16, tag="kt")       # K = W2 x + b2
        qt = sb_kq.tile([C, NQ], F16, tag="qt")      # Q = W1 x + b1 (query half)

        def proj(dst, w_ofs, b_col, x_ofs, j, act_evac, halves=1):
            ps = ps_r.tile([C, QB], F32, tag="r")
            nc.tensor.matmul(ps[:], xw[:, w_ofs:w_ofs + C],
                             xw[:, x_ofs + j * QB:x_ofs + (j + 1) * QB],
                             start=True, stop=True)
            if act_evac:
                nc.scalar.activation(
                    dst[:, j * QB:(j + 1) * QB], ps[:],
                    mybir.ActivationFunctionType.Identity,
                    bias=xr[:, NQ + b_col:NQ + b_col + 1])
                return
            h = QB // halves
            for s0 in range(halves):
                nc.vector.tensor_scalar(
                    out=dst[:, j * QB + s0 * h:j * QB + (s0 + 1) * h],
                    in0=ps[:, s0 * h:(s0 + 1) * h],
                    scalar1=xr[:, NQ + b_col:NQ + b_col + 1], scalar2=None,
                    op0=mybir.AluOpType.add)

        # prologue: only the projections the first S-matmuls need; the
        # rest sprinkle into pass 0 so the exp stream starts ~10us earlier
        proj(qt, W1T_OFS, 0, XQ_OFS, 0, True)
        proj(kt, W2T_OFS, 1, XK_OFS, 0, False)
        proj(qt, W1T_OFS, 0, XQ_OFS, 1, True)
        # K-chunk c feeds S-matmuls from m-tile 4c; sprinkle it at 2(c-1)
        sprinkle = {0: (kt, W2T_OFS, 1, XK_OFS, 1), 2: (kt, W2T_OFS, 1, XK_OFS, 2),
                    4: (kt, W2T_OFS, 1, XK_OFS, 3), 6: (kt, W2T_OFS, 1, XK_OFS, 4),
                    8: (kt, W2T_OFS, 1, XK_OFS, 5), 10: (kt, W2T_OFS, 1, XK_OFS, 6),
                    12: (kt, W2T_OFS, 1, XK_OFS, 7), 14: (qt, W1T_OFS, 0, XQ_OFS, 2),
                    16: (qt, W1T_OFS, 0, XQ_OFS, 3)}

        # E staged for a whole pass in SBUF so O-matmuls can lag
        e_stage = sb_e.tile([C, N_MT * 2 * QB], BF16, tag="e")

        # two passes, each covering a pair of query blocks (2*QB = 1024 q)
        for p in range(N_QB // 2):
            q0 = 2 * p * QB                      # col offset of this q-pair
            o_psA = ps_o.tile([C, QB], F32, tag="o")
            o_psB = ps_o.tile([C, QB], F32, tag="o")
            acc = sb_acc.tile([C, 2 * QB], BF16, tag="acc")

            def do_s(mt):
                s_ps = ps_s.tile([C, 2 * QB], F32, tag="s")
                for j in range(2):
                    nc.tensor.matmul(
                        s_ps[:, j * QB:(j + 1) * QB],
                        kt[:, mt * MT:(mt + 1) * MT],
                        qt[:, q0 + j * QB:q0 + (j + 1) * QB],
                        start=True, stop=True)
                e_g = e_stage[:, mt * 2 * QB:(mt + 1) * 2 * QB]
                nc.scalar.activation(e_g, s_ps[:],
                                     mybir.ActivationFunctionType.Exp,
                                     bias=shift[:, 0:1])
                if mt == 0:
                    nc.vector.tensor_copy(acc[:], e_g)
                elif mt < N_MT - 2:
                    # last two m-tiles skip the DVE accumulate: the row-sum
                    # matmul picks them up directly from e_stage, so the
                    # reciprocal can start right after the final exp
                    nc.vector.tensor_tensor(acc[:], acc[:], e_g,
                                            op=mybir.AluOpType.add)

            def do_o(mt):
                for j, o_ps in enumerate((o_psA, o_psB)):
                    nc.tensor.matmul(
                        o_ps[:],
                        xt[:, mt * MT:(mt + 1) * MT],
                        e_stage[:, (mt * 2 + j) * QB:(mt * 2 + j + 1) * QB],
                        start=(mt == 0), stop=(mt == N_MT - 1))

            r_tiles = []
            for mt in range(N_MT + DLAG):
                if p == 0 and mt in sprinkle:
                    dst, w_ofs, b_col, x_ofs, j = sprinkle[mt]
                    proj(dst, w_ofs, b_col, x_ofs, j, False, halves=2)
                if mt < N_MT:
                    do_s(mt)
                if mt >= DLAG:
                    do_o(mt - DLAG)
                # 3-stage row-sum accumulation interleaved with trailing O's:
                # stage 1 reads acc (complete through m-tile 29), stages 2-3
                # read the last two E-tiles straight from the stage buffer
                if mt == N_MT:
                    for j in range(2):
                        rp = ps_r.tile([C, QB], F32, tag="r")
                        nc.tensor.matmul(rp[:], ones_bf[:],
                                         acc[:, j * QB:(j + 1) * QB],
                                         start=True, stop=False)
                        r_tiles.append(rp)
                if mt in (N_MT + 1, N_MT + 2):
                    emt = N_MT - 2 + (mt - N_MT - 1)
                    for j in range(2):
                        nc.tensor.matmul(
                            r_tiles[j][:], ones_bf[:],
                            e_stage[:, (emt * 2 + j) * QB:(emt * 2 + j + 1) * QB],
                            start=False, stop=(mt == N_MT + 2))

            # per-qb tail: row-sum -> reciprocal -> broadcast -> norm+residual+relu
            for j, o_ps in enumerate((o_psA, o_psB)):
                qofs = q0 + j * QB
                bc = sb_tail.tile([C, QB], F32, tag="bcs")
                nc.vector.reciprocal_approx_fast(bc[:], r_tiles[j][:])
                t2 = sb_tail.tile([C, QB], F32, tag="t2")
                nc.vector.tensor_tensor(t2[:], o_ps[:], bc[:],
                                        op=mybir.AluOpType.mult)
                t3 = sb_tail.tile([C, QB], F32, tag="t3")
                nc.vector.tensor_tensor(t3[:], t2[:], xr[:, qofs:qofs + QB],
                                        op=mybir.AluOpType.add)
                o_out = sb_tail.tile([C, QB], F32, tag="o_out")
                nc.vector.tensor_scalar_max(o_out[:], t3[:], 0.0)
                nc.sync.dma_start(out_ext[:, qofs:qofs + QB], o_out[:])

    nc.compile()
    return nc


_NC_CACHE = None


def _get_nc():
    global _NC_CACHE
    if _NC_CACHE is None:
        _NC_CACHE = build_nc()
    return _NC_CACHE


def make_in_maps(x, W1, b1, W2, b2):
    x = np.asarray(x, np.float32)
    W1 = np.asarray(W1, np.float32)
    b1 = np.asarray(b1, np.float32)
    W2 = np.asarray(W2, np.float32)
    b2 = np.asarray(b2, np.float32)
    in_maps = []
    for core in range(8):
        b, h = divmod(core, 2)
        xb = x[b]                                    # [128, 4096]
        xq = xb[:, h * NQ:(h + 1) * NQ]
        xw = np.empty((C, XW_COLS), np.float16)
        xw[:, XK_OFS:XK_OFS + N] = xb
        xw[:, XQ_OFS:XQ_OFS + NQ] = xq
        xw[:, W1T_OFS:W1T_OFS + C] = W1.T
        xw[:, W2T_OFS:W2T_OFS + C] = W2.T
        xw[:, B1_OFS] = b1
        xw[:, B2_OFS] = b2
        # xt[:, mt*128 + c] = x[b].T[mt*128 + (partition), c]
        xt = np.ascontiguousarray(
            xb.T.reshape(N_MT, MT, C).transpose(1, 0, 2).reshape(MT, N_MT * C)
        ).astype(ml_dtypes.bfloat16)
        xrr = np.empty((C, NQ + 2), np.float32)
        xrr[:, :NQ] = xq
        xrr[:, NQ] = b1
        xrr[:, NQ + 1] = b2
        in_maps.append({"xw": xw, "xt": xt, "xr": xrr})
    return in_maps


def run(x, W1, b1, W2, b2, trace=False):
    nc = _get_nc()
    in_maps = make_in_maps(x, W1, b1, W2, b2)
    last_err = None
    for _attempt in range(3):
        try:
            res = run_bass_kernel_spmd(nc, in_maps, core_ids=list(range(8)),
                                       trace=trace)
            break
        except Exception as e:  # transient NRT/device errors: retry
            last_err = e
    else:
        raise last_err
    out = np.empty((B, C, N), np.float32)
    for core in range(8):
        b, h = divmod(core, 2)
        out[b][:, h * NQ:(h + 1) * NQ] = res.results[core]["out"]
    return out, res


def kernel(x, W1, b1, W2, b2):
    out, _ = run(x, W1, b1, W2, b2, trace=False)
    return out



# revision 6
# speedup vs baseline: 1.2339x; 1.2339x over previous
"""Trainium2 Bass kernel for nn_Attention_4063039062503.

Reference (per batch b, C=128 channels, N=4096 points):
    q = W1 @ x + b1;  k = W2 @ x + b2          # [C, N]
    s[n, m] = q[:, n] . k[:, m]                # [N, N]
    a = softmax(s, axis=m)
    out = relu(x + x @ a.T)

Math restructure (the projections collapse into one tiny GEMM):
    KtQ = x_K^T (W2^T W1) x_q + u 1^T + 1 v^T + const,  u = x_K^T (W2^T b1)
    The v/const terms are constant over keys -> cancel in softmax.
    So  S_eff^T = x_K^T Z'   with   Z' = A^T... Z' = (W2^T W1) x_q + w 1^T,
    w = W2^T b1.  Host precomputes A = W1^T W2 (lhsT of the Z-proj) and w;
    the device does Z' = matmul(A, x_q) + w (bias folded into the PSUM
    evacuation) and never touches W1/W2/b1/b2 again.  exp() then needs only
    a constant -30 shift -> the ACT engine does nothing but 64 pure exps.

Sharding: 8 cores, core i -> batch i//2, query half i%2 (2048 queries),
full 4096 keys local (no collectives).  Keys are ROTATED per core so the
query half is always columns 0:2048 -> one ascending DMA stream feeds the
Z-projection and the early S-tiles.

Per-core pipeline (flash-attention style, flattened 64-iteration stream):
  - S^T tile [m=128, 1024 q] per (pass, m-tile) on TensorE in fp16,
    single 1024-free matmul
  - exp(s - 30) on ScalarE PSUM->SBUF, bf16 out; ACT runs back-to-back
  - O[c, q] += xT[m-tile] @ E on TensorE bf16, fp32 PSUM accumulation
  - row-sums: DVE bf16 accumulate + 3-stage ones-matmul (stages 2-3 read
    the last two E-tiles directly so the reciprocal starts immediately)
  - pass-0 uses ps_o for O and ps_r for rowsum; pass-1 SWAPS them (O in
    the 2-bank [C,1024] r-slot, rowsums in the o-slots) so pass-1's first
    O-matmul never waits on pass-0's tail reads
  - tail: reciprocal_approx_fast + normalize + residual on DVE; final
    relu on ACT (free after the last exp), DMA out
"""
from contextlib import ExitStack

import numpy as np
import ml_dtypes

import concourse.tile as tile
from concourse import bacc, mybir
from concourse.bass_utils import run_bass_kernel_spmd

B = 4
C = 128
N = 4096            # keys per batch
NQ = 2048           # queries per core
PW = 1024           # queries per pass
MT = 128            # m (key) tile
N_MT = N // MT      # 32
N_P = NQ // PW      # 2 passes
TOT = N_P * N_MT    # 64 global iterations
DLAG = 3            # O-matmul lag (in m-tiles) behind the S/exp stream
MMF = 512           # max matmul free size (1024 is rejected by the ISA:
                    # a matmul's PSUM output cannot cross a 2KB bank)

F32 = mybir.dt.float32
F16 = mybir.dt.float16
BF16 = mybir.dt.bfloat16
AF = mybir.ActivationFunctionType
ALU = mybir.AluOpType


def build_nc():
    nc = bacc.Bacc("TRN2", target_bir_lowering=False, debug=False, num_devices=8)
    a_ext = nc.declare_dram_parameter("a16", [C, C], F16, isOutput=False)
    xk_ext = nc.declare_dram_parameter("xk", [C, N], F16, isOutput=False)
    xt_ext = nc.declare_dram_parameter("xt", [C, N], BF16, isOutput=False)
    # col 0 = w (Z-proj bias), cols 1..NQ = residual x_q
    xr_ext = nc.declare_dram_parameter("xr", [C, NQ + 1], F32, isOutput=False)
    out_ext = nc.declare_dram_parameter("out", [C, NQ], F32, isOutput=True)

    def mm(out_ap, lhsT, rhs, start=True, stop=True):
        wtot = out_ap.shape[-1]
        o = 0
        while o < wtot:
            wd = min(MMF, wtot - o)
            nc.tensor.matmul(out_ap[:, o:o + wd], lhsT, rhs[:, o:o + wd],
                             start=start, stop=stop)
            o += wd

    with ExitStack() as ctx:
        tc = ctx.enter_context(tile.TileContext(nc))
        consts = ctx.enter_context(tc.tile_pool(name="consts", bufs=1))
        sb_in = ctx.enter_context(tc.tile_pool(name="sb_in", bufs=1))
        sb_z = ctx.enter_context(tc.tile_pool(name="sb_z", bufs=1))
        sb_e = ctx.enter_context(tc.tile_pool(name="sb_e", bufs=1))
        sb_acc = ctx.enter_context(tc.tile_pool(name="sb_acc", bufs=2))
        sb_tail = ctx.enter_context(tc.tile_pool(name="sb_tail", bufs=2))
        ps_s = ctx.enter_context(tc.tile_pool(name="ps_s", bufs=2, space="PSUM"))
        ps_o = ctx.enter_context(tc.tile_pool(name="ps_o", bufs=2, space="PSUM"))
        ps_r = ctx.enter_context(tc.tile_pool(name="ps_r", bufs=1, space="PSUM"))

        # warm the PE's HAM clock gate (~3.4us of matmul activity) during
        # the input-DMA wait so the main stream runs at 2.4 GHz; its
        # memset goes first so the warmup starts before the first DMA lands
        wmm = consts.tile([C, 512], BF16, tag="wmm")
        nc.vector.memset(wmm[:], 0.0)
        for _ in range(8):
            wps = ps_s.tile([C, PW], F32, tag="s")
            nc.tensor.matmul(wps[:, 0:512], wmm[:, 0:C], wmm[:],
                             start=True, stop=True)
        ones_bf = consts.tile([C, C], BF16, tag="ones_bf")
        nc.vector.memset(ones_bf[:], 1.0)
        shift = consts.tile([C, 1], F32, tag="shift")
        nc.vector.memset(shift[:], -30.0)
        # warm the exp table early (ACT_TABLE_LOAD ~2.7us)
        warm = consts.tile([1, 16], F32, tag="warm")
        nc.vector.memset(warm[:], 0.0)
        warm_o = consts.tile([1, 16], F32, tag="warm_o")
        nc.scalar.activation(warm_o[:], warm[:], AF.Exp)

        a16 = sb_in.tile([C, C], F16, tag="a16")
        xk = sb_in.tile([C, N], F16, tag="xk")
        xt = sb_in.tile([C, N], BF16, tag="xt")
        xr = sb_in.tile([C, NQ + 1], F32, tag="xr")
        zt = sb_z.tile([C, NQ], F16, tag="zt")
        e_stage = sb_e.tile([C, N_MT * PW], BF16, tag="e")

        # input DMAs, gating-first order: the first exp needs only
        # a16 + w + xk[:, 0:PW]
        nc.sync.dma_start(a16[:], a_ext[:])
        nc.sync.dma_start(xr[:, 0:1], xr_ext[:, 0:1])
        nc.sync.dma_start(xk[:, 0:PW], xk_ext[:, 0:PW])
        nc.sync.dma_start(xt[:, 0:PW], xt_ext[:, 0:PW])
        nc.sync.dma_start(xk[:, PW:2 * PW], xk_ext[:, PW:2 * PW])
        nc.sync.dma_start(xt[:, PW:2 * PW], xt_ext[:, PW:2 * PW])
        nc.sync.dma_start(xk[:, 2 * PW:N], xk_ext[:, 2 * PW:N])
        nc.sync.dma_start(xt[:, 2 * PW:N], xt_ext[:, 2 * PW:N])
        nc.sync.dma_start(xr[:, 1:NQ + 1], xr_ext[:, 1:NQ + 1])

        def zproj(j, split_evac):
            # Z'[:, j*PW:(j+1)*PW] = A^T... = (W2^T W1) x_q + w 1^T
            zp = ps_r.tile([C, PW], F32, tag="r")
            mm(zp[:], a16[:], xk[:, j * PW:(j + 1) * PW])
            dst0 = zt[:, j * PW:j * PW + 512]
            dst1 = zt[:, j * PW + 512:(j + 1) * PW]
            nc.vector.tensor_scalar(out=dst0, in0=zp[:, 0:512],
                                    scalar1=xr[:, 0:1], scalar2=None,
                                    op0=ALU.add)
            if split_evac:
                # second half on ACT so the first S-matmul starts sooner
                nc.scalar.activation(dst1, zp[:, 512:PW], AF.Identity,
                                     bias=xr[:, 0:1])
            else:
                nc.vector.tensor_scalar(out=dst1, in0=zp[:, 512:PW],
                                        scalar1=xr[:, 0:1], scalar2=None,
                                        op0=ALU.add)

        zproj(0, split_evac=True)

        acc = [None] * N_P
        o_t = [None] * N_P      # pass 0: (o_psA, o_psB); pass 1: [C,1024]
        r_t = [None] * N_P      # pass 0: [C,1024];       pass 1: (rA, rB)

        def do_o(gg):
            p, mt = divmod(gg, N_MT)
            st = (mt == 0)
            sp = (mt == N_MT - 1)
            if p == 0:
                if st:
                    o_t[0] = (ps_o.tile([C, 512], F32, tag="o", name="o0a"),
                              ps_o.tile([C, 512], F32, tag="o", name="o0b"))
                for j in range(2):
                    nc.tensor.matmul(
                        o_t[0][j][:], xt[:, mt * MT:(mt + 1) * MT],
                        e_stage[:, mt * PW + j * 512:mt * PW + (j + 1) * 512],
                        start=st, stop=sp)
            else:
                if st:
                    o_t[1] = ps_r.tile([C, PW], F32, tag="r", name="o1")
                mm(o_t[1][:], xt[:, mt * MT:(mt + 1) * MT],
                   e_stage[:, mt * PW:(mt + 1) * PW], start=st, stop=sp)

        def rstage(p, stage):
            # 3-stage row-sum: stage 0 reads acc (complete through m-tile
            # 29), stages 1-2 read the last two E-tiles straight from the
            # stage buffer so the reciprocal can start right after the
            # final exp of the pass
            st = (stage == 0)
            sp = (stage == 2)
            if stage == 0:
                rhs = acc[p][:]
            else:
                emt = N_MT - 3 + stage  # 30, 31
                rhs = e_stage[:, emt * PW:(emt + 1) * PW]
            if p == 0:
                if st:
                    r_t[0] = ps_r.tile([C, PW], F32, tag="r", name="r0")
                mm(r_t[0][:], ones_bf[:], rhs, start=st, stop=sp)
            else:
                if st:
                    r_t[1] = (ps_o.tile([C, 512], F32, tag="o", name="r1a"),
                              ps_o.tile([C, 512], F32, tag="o", name="r1b"))
                for j in range(2):
                    nc.tensor.matmul(r_t[1][j][:], ones_bf[:],
                                     rhs[:, j * 512:(j + 1) * 512],
                                     start=st, stop=sp)

        def tail(p):
            # bc = 1/rowsum; out = relu(O*bc + x); pass-0 relu on DVE
            # (ACT is saturated with exps), pass-1 relu on ACT (free)
            bc = sb_tail.tile([C, PW], F32, tag="bc")
            if p == 0:
                nc.vector.reciprocal_approx_fast(bc[:], r_t[0][:])
            else:
                for j in range(2):
                    nc.vector.reciprocal_approx_fast(
                        bc[:, j * 512:(j + 1) * 512], r_t[1][j][:])
            t2 = sb_tail.tile([C, PW], F32, tag="t2")
            if p == 0:
                for j in range(2):
                    nc.vector.tensor_tensor(t2[:, j * 512:(j + 1) * 512],
                                            o_t[0][j][:],
                                            bc[:, j * 512:(j + 1) * 512],
                                            op=ALU.mult)
            else:
                nc.vector.tensor_tensor(t2[:], o_t[1][:], bc[:], op=ALU.mult)
            t3 = sb_tail.tile([C, PW], F32, tag="t3")
            oo = sb_tail.tile([C, PW], F32, tag="oo")
            for j in range(2):
                sl = slice(j * 512, (j + 1) * 512)
                nc.vector.tensor_tensor(t3[:, sl], t2[:, sl],
                                        xr[:, 1 + p * PW + j * 512:
                                            1 + p * PW + (j + 1) * 512],
                                        op=ALU.add)
                if p == 0:
                    nc.vector.tensor_scalar_max(oo[:, sl], t3[:, sl], 0.0)
                else:
                    nc.scalar.activation(oo[:, sl], t3[:, sl], AF.Relu)
                nc.sync.dma_start(out_ext[:, p * PW + j * 512:
                                          p * PW + (j + 1) * 512], oo[:, sl])

        for g in range(TOT + DLAG):
            p, mt = divmod(g, N_MT)
            if g < TOT:
                s_ps = ps_s.tile([C, PW], F32, tag="s")
                mm(s_ps[:], xk[:, mt * MT:(mt + 1) * MT],
                   zt[:, p * PW:(p + 1) * PW])
                e_g = e_stage[:, mt * PW:(mt + 1) * PW]
                nc.scalar.activation(e_g, s_ps[:], AF.Exp, bias=shift[:, 0:1])
            # row-sum stages for the pass ending at g-?: emitted right after
            # this iteration's S so they never delay the next exp
            for pp in range(N_P):
                base = pp * N_MT + N_MT - 1  # g of the pass's S(31)
                if g == base:
                    rstage(pp, 0)
                elif g == base + 1:
                    rstage(pp, 1)
                elif g == base + 2:
                    rstage(pp, 2)
            if g == 1:
                zproj(1, split_evac=False)
            if g < TOT:
                p, mt = divmod(g, N_MT)
                # DVE row-sum accumulation (m-tiles 0..29; last two are
                # picked up directly by rstages 1-2)
                if mt == 1:
                    acc[p] = sb_acc.tile([C, PW], BF16, tag="acc", name=f"acc{p}")
                    nc.vector.tensor_tensor(acc[p][:], e_stage[:, 0:PW],
                                            e_g, op=ALU.add)
                elif 2 <= mt <= N_MT - 3:
                    nc.vector.tensor_tensor(acc[p][:], acc[p][:], e_g,
                                            op=ALU.add)
            if g >= DLAG:
                do_o(g - DLAG)
            for pp in range(N_P):
                # after do_o(pp, 31): the O accumulator is complete
                if g == pp * N_MT + N_MT - 1 + DLAG:
                    tail(pp)

    nc.compile()
    return nc


_NC_CACHE = None


def _get_nc():
    global _NC_CACHE
    if _NC_CACHE is None:
        _NC_CACHE = build_nc()
    return _NC_CACHE


def make_in_maps(x, W1, b1, W2, b2):
    x = np.asarray(x, np.float32)
    W1 = np.asarray(W1, np.float32)
    b1 = np.asarray(b1, np.float32)
    W2 = np.asarray(W2, np.float32)
    b2 = np.asarray(b2, np.float32)
    A = (W1.T @ W2).astype(np.float16)          # lhsT of the Z-projection
    w = W2.T @ b1                               # folded u-bias
    in_maps = []
    for core in range(8):
        b, h = divmod(core, 2)
        xb = x[b]                               # [128, 4096]
        # rotate keys so this core's query half is columns 0:2048
        xrot = np.concatenate([xb[:, h * NQ:], xb[:, :h * NQ]], axis=1)
        xk16 = xrot.astype(np.float16)
        # xt[m, mt*128 + c] = xrot[c, mt*128 + m]
        xtt = np.ascontiguousarray(
            xrot.T.reshape(N_MT, MT, C).transpose(1, 0, 2).reshape(MT, N_MT * C)
        ).astype(ml_dtypes.bfloat16)
        xrr = np.empty((C, NQ + 1), np.float32)
        xrr[:, 0] = w
        xrr[:, 1:NQ + 1] = xrot[:, :NQ]
        in_maps.append({"a16": A, "xk": xk16, "xt": xtt, "xr": xrr})
    return in_maps


def run(x, W1, b1, W2, b2, trace=False):
    nc = _get_nc()
    in_maps = make_in_maps(x, W1, b1, W2, b2)
    last_err = None
    for _attempt in range(3):
        try:
            res = run_bass_kernel_spmd(nc, in_maps, core_ids=list(range(8)),
                                       trace=trace)
            break
        except Exception as e:  # transient NRT/device errors: retry
            last_err = e
    else:
        raise last_err
    out = np.empty((B, C, N), np.float32)
    for core in range(8):
        b, h = divmod(core, 2)
        out[b][:, h * NQ:(h + 1) * NQ] = res.results[core]["out"]
    return out, res


def kernel(x, W1, b1, W2, b2):
    out, _ = run(x, W1, b1, W2, b2, trace=False)
    return out
